# revision 26
# baseline (speedup 1.0000x reference)
"""CertViT (ViT-Base + layer-3 token pruning) forward pass on 8 Trainium2 cores.

Data parallel: 8 images per core, processed as 4 image-pairs so matmul free
dims (394 / 278) stay >= 256 for full-rate fp32r. Activations live in
channel-partition layout x^T [768 -> 6x128 chunks, tokens]. LayerNorm affine
params are folded into the following matmul weights on the host; device LN is
pure standardization using ones-matmul partition broadcasts. Top-k pruning
uses max8/match_replace for the drop mask, a triangular-matmul cumsum for
ranks, and a one-hot permutation matmul for the gather.
"""

import os
import sys

import numpy as np

for _p in ('/opt/trn_rl_repo', '/root/.axon_site/_ro/trn_rl_repo'):
    if os.path.isdir(_p) and _p not in sys.path:
        sys.path.append(_p)

import concourse.bass as bass
import concourse.mybir as mybir
from concourse.tile import TileContext
from concourse.bass_utils import run_bass_kernel_spmd
from concourse.alu_op_type import AluOpType as AL

dt = mybir.dt
AF = mybir.ActivationFunctionType

# ---------------------------------------------------------------- config
NCORES = 8
B_CORE = 8            # images per core
PAIRS = B_CORE // 2
C = 768
CH = C // 128          # 6 channel chunks
HD = 12                # heads
D = 64                 # head dim
SCALE = D ** -0.5
DEPTH = 12
SEL = 3                # pruning layer
N0 = 197               # tokens before pruning
K_KEEP = 137           # int(197*0.7)
N_DROP = N0 - 1 - K_KEEP   # 59
N1 = K_KEEP + 2        # 139 tokens after pruning
F0 = 2 * N0            # pair free dim, layers 0..3
F1 = 2 * N1            # pair free dim, layers 4..11
EPS = 1e-6
NCLS = 100

# ------------------------------------------------------------- waitfix
# This walrus build accepts at most ONE sem wait per instruction; Tile can
# attach several. Move excess waits onto InstNoOp carriers inserted before.
_wf_counter = [0]


def _wf_carrier(engine, waits):
    _wf_counter[0] += 1
    d = mybir.InstNoOp(name=f"waitfix-{_wf_counter[0]}", ins=[], outs=[])
    d.engine = engine
    d.sync_info = mybir.SyncInfo(on_wait=list(waits), on_update=[])
    return d


def split_excess_waits(nc, max_waits=1):
    nfix = 0
    for f in nc.m.functions:
        for bb in f.blocks:
            insts = list(bb.instructions)
            out = []
            changed = False
            for inst in insts:
                si = inst.sync_info
                waits = list(si.on_wait) if si and si.on_wait else []
                if len(waits) > max_waits:
                    keep, rest = waits[:max_waits], waits[max_waits:]
                    while rest:
                        chunk, rest = rest[:max_waits], rest[max_waits:]
                        out.append(_wf_carrier(inst.engine, chunk))
                    si.on_wait = keep
                    changed = True
                    nfix += 1
                out.append(inst)
            if changed:
                bb.instructions = out
    return nfix


# ----------------------------------------------------------- device kernel
def build_nc():
    nc = bass.Bass()
    f32, f32r = dt.float32, dt.float32r

    d = {}
    d["patches_d"] = nc.declare_dram_parameter("patchesT", [C, B_CORE * 196], f32r, isOutput=False)
    d["posc_d"] = nc.declare_dram_parameter("posCT", [C, N0], f32, isOutput=False)
    d["pw_d"] = nc.declare_dram_parameter("patch_wT", [C, C], f32r, isOutput=False)
    d["qkvw_d"] = nc.declare_dram_parameter("qkv_wT", [DEPTH, C, 3 * C], f32r, isOutput=False)
    d["qkvb_d"] = nc.declare_dram_parameter("qkv_bL", [DEPTH, 128, 18], f32, isOutput=False)
    d["projw_d"] = nc.declare_dram_parameter("proj_wT", [DEPTH, C, C], f32r, isOutput=False)
    d["projb_d"] = nc.declare_dram_parameter("proj_bL", [DEPTH, 128, 6], f32, isOutput=False)
    d["fc1w_d"] = nc.declare_dram_parameter("fc1_wT", [DEPTH, C, 4 * C], f32r, isOutput=False)
    d["fc1b_d"] = nc.declare_dram_parameter("fc1_bL", [DEPTH, 128, 24], f32, isOutput=False)
    d["fc2w_d"] = nc.declare_dram_parameter("fc2_wT", [DEPTH, 4 * C, C], f32r, isOutput=False)
    d["fc2b_d"] = nc.declare_dram_parameter("fc2_bL", [DEPTH, 128, 6], f32, isOutput=False)
    d["headw_d"] = nc.declare_dram_parameter("headT", [C, NCLS], f32r, isOutput=False)
    d["headb_d"] = nc.declare_dram_parameter("head_bL", [NCLS, 1], f32, isOutput=False)
    d["ident_d"] = nc.declare_dram_parameter("ident", [128, 128], f32, isOutput=False)
    d["ones_d"] = nc.declare_dram_parameter("ones", [128, 128], f32r, isOutput=False)
    d["invc_d"] = nc.declare_dram_parameter("invC", [128, 128], f32r, isOutput=False)
    d["iota_d"] = nc.declare_dram_parameter("iota", [128, N1 - 1], f32, isOutput=False)
    d["lt_d"] = nc.declare_dram_parameter("LT", [196, 196], f32r, isOutput=False)
    d["out_d"] = nc.declare_dram_parameter("logitsT", [NCLS, B_CORE], f32, isOutput=True)

    d["dbg_layer"] = os.environ.get("BASS_VIT_DEBUG_LAYER", "")
    if d["dbg_layer"]:
        d["dbg_d"] = nc.declare_dram_parameter("dbg", [1 + 2 * DEPTH, 128, CH * F0], f32, isOutput=True)
        d["dbgp_d"] = nc.declare_dram_parameter("dbgp", [4, 8, 196], f32, isOutput=True)
    else:
        d["dbg_d"] = None
        d["dbgp_d"] = None

    with TileContext(nc) as tc:
        _build_body(nc, tc, d)
    return nc


def _build_body(nc, tc, d):
    f32, f32r = dt.float32, dt.float32r
    from contextlib import ExitStack
    es = ExitStack()

    cpool = es.enter_context(tc.tile_pool(name="consts", bufs=1))
    xpool = es.enter_context(tc.tile_pool(name="x", bufs=1))
    ppool = es.enter_context(tc.tile_pool(name="psum", bufs=1, space="PSUM"))
    prpool = es.enter_context(tc.tile_pool(name="prune", bufs=1))
    bpool = es.enter_context(tc.tile_pool(name="bias", bufs=2))

    # constants
    ident = cpool.tile([128, 128], f32, tag="ident")
    ones = cpool.tile([128, 128], f32r, tag="ones")
    invc = cpool.tile([128, 128], f32r, tag="invc")
    iota = cpool.tile([128, N1 - 1], f32, tag="iota")
    ltt = cpool.tile([128, 2 * 196], f32r, tag="ltt")
    posct = cpool.tile([128, CH * N0], f32, tag="posct")
    eps_t = cpool.tile([128, 1], f32, tag="eps_t")
    nc.vector.memset(eps_t[:], EPS)
    nc.sync.dma_start(ident[:], d["ident_d"][:])
    nc.sync.dma_start(ones[:], d["ones_d"][:])
    nc.sync.dma_start(invc[:], d["invc_d"][:])
    nc.sync.dma_start(iota[:], d["iota_d"][:])
    nc.sync.dma_start(ltt[:, 0:196], d["lt_d"][0:128, :])
    nc.sync.dma_start(ltt[0:68, 196:392], d["lt_d"][128:196, :])
    nc.sync.dma_start(posct[:].rearrange("p (k n) -> p k n", k=CH), d["posc_d"].rearrange("(k p) n -> p k n", p=128))

    # PSUM slots: tag 'a' x4 (main accumulations), 'b' x2, 'c' x2 -> 8 banks
    def psA():
        return ppool.tile([128, F0], f32, tag="a", bufs=4, name="psA")

    def psB():
        return ppool.tile([128, F0], f32, tag="b", bufs=2, name="psB")

    def psC():
        return ppool.tile([128, F0], f32, tag="c", bufs=2, name="psC")

    # persistent per-pair residual stream x^T, chunk-major [128, CH*F]
    xt = [xpool.tile([128, CH * F0], f32r, tag=f"x{p}", name=f"x{p}") for p in range(PAIRS)]
    # per-pair uncertainty rows (filled at layer SEL)
    unc = [prpool.tile([1, F0], f32, tag=f"unc{p}", name=f"unc{p}") for p in range(PAIRS)]

    # ------------------------------------------------------------ patch embed
    with tc.tile_pool(name="wpatch", bufs=1) as wp, tc.tile_pool(name="tpatch", bufs=2) as tp:
        pwt = wp.tile([128, CH * C], f32r, tag="pw")
        nc.sync.dma_start(pwt[:].rearrange("p (k n) -> p k n", k=CH), d["pw_d"].rearrange("(k p) n -> p k n", p=128))
        for p in range(PAIRS):
            prt = tp.tile([128, CH * 392], f32r, tag="patches")
            nc.sync.dma_start(
                prt[:].rearrange("p (k n) -> p k n", k=CH),
                d["patches_d"][:, p * 392:(p + 1) * 392].rearrange("(k p) n -> p k n", p=128),
            )
            for co in range(CH):
                ps = psA()
                for k in range(CH):
                    nc.tensor.matmul(
                        ps[:, 0:392],
                        pwt[:, k * C + co * 128: k * C + co * 128 + 128],
                        prt[:, k * 392:(k + 1) * 392],
                        start=(k == 0), stop=(k == CH - 1),
                    )
                for b in range(2):
                    nc.vector.tensor_tensor(
                        xt[p][:, co * F0 + b * N0 + 1: co * F0 + b * N0 + N0],
                        ps[:, b * 196:(b + 1) * 196],
                        posct[:, co * N0 + 1: co * N0 + N0],
                        op=AL.add,
                    )
                    nc.vector.tensor_copy(
                        xt[p][:, co * F0 + b * N0: co * F0 + b * N0 + 1],
                        posct[:, co * N0: co * N0 + 1],
                    )

    def tap(slot, xtile, F):
        if d["dbg_d"] is not None:
            nc.sync.dma_start(d["dbg_d"][slot][:, 0:CH * F], xtile[:, 0:CH * F].bitcast(f32))

    tap(0, xt[0], F0)

    # ------------------------------------------------------------ helpers
    def layernorm(pool, x, F, xh_tag, xh_bufs=1):
        """Standardize x (chunk-major [128, CH*F]) per token -> fp32r tile."""
        xh = pool.tile([128, CH * F], f32r, tag=xh_tag, bufs=xh_bufs, name=xh_tag)
        sq = pool.tile([128, CH * F], f32r, tag="ln_sq", bufs=1)
        for k in range(CH):
            nc.vector.tensor_tensor(
                sq[:, k * F:(k + 1) * F],
                x[:, k * F:(k + 1) * F].bitcast(f32),
                x[:, k * F:(k + 1) * F].bitcast(f32),
                op=AL.mult,
            )
        pm = psB()
        ps2 = psC()
        for k in range(CH):
            nc.tensor.matmul(pm[:, 0:F], invc[:], x[:, k * F:(k + 1) * F],
                             start=(k == 0), stop=(k == CH - 1))
        for k in range(CH):
            nc.tensor.matmul(ps2[:, 0:F], invc[:], sq[:, k * F:(k + 1) * F],
                             start=(k == 0), stop=(k == CH - 1))
        var = pool.tile([128, F], f32, tag="ln_var", bufs=1)
        rstd = pool.tile([128, F], f32, tag="ln_rstd", bufs=1)
        mean = pool.tile([128, F], f32, tag="ln_mean", bufs=1)
        nc.vector.tensor_copy(mean[:], pm[:, 0:F])
        nc.vector.tensor_tensor(var[:], mean[:], mean[:], op=AL.mult)
        nc.vector.tensor_tensor(var[:], ps2[:, 0:F], var[:], op=AL.subtract)
        nc.scalar.activation(rstd[:], var[:], AF.Sqrt, bias=eps_t[:, 0:1])
        nc.vector.reciprocal(rstd[:], rstd[:])
        for k in range(CH):
            nc.vector.tensor_tensor(
                var[:], x[:, k * F:(k + 1) * F].bitcast(f32), mean[:], op=AL.subtract)
            nc.vector.tensor_tensor(
                xh[:, k * F:(k + 1) * F], var[:], rstd[:], op=AL.mult)
        return xh

    def load_bias(dram_t, l, cols):
        bt = bpool.tile([128, cols], f32, tag=dram_t.name)
        nc.sync.dma_start(bt[:], dram_t[l])
        return bt

    # ------------------------------------------------------------ layers
    for l in range(DEPTH):
        F = F0 if l <= SEL else F1
        N = N0 if l <= SEL else N1
        mlens = [128, N - 128]

        qkvb = load_bias(d["qkvb_d"], l, 18)
        projb = load_bias(d["projb_d"], l, 6)

        # ---------------- phase A: LN1 + QKV + attention + proj ----------------
        with tc.tile_pool(name="wA", bufs=1) as wA, tc.tile_pool(name="tA", bufs=1) as tA:
            wq = wA.tile([128, CH * 3 * C], f32r, tag="wqkv")
            nc.sync.dma_start(wq[:].rearrange("p (k n) -> p k n", k=CH), d["qkvw_d"][l].rearrange("(k p) n -> p k n", p=128))
            wpj = wA.tile([128, CH * C], f32r, tag="wproj")
            nc.sync.dma_start(wpj[:].rearrange("p (k n) -> p k n", k=CH), d["projw_d"][l].rearrange("(k p) n -> p k n", p=128))

            for p in range(PAIRS):
                xh = layernorm(tA, xt[p], F, "ln1")
                qT = tA.tile([128, CH * F], f32r, tag="qT")
                kT = tA.tile([128, CH * F], f32r, tag="kT")
                for o in range(12):
                    ps = psA()
                    for k in range(CH):
                        nc.tensor.matmul(
                            ps[:, 0:F],
                            wq[:, k * 3 * C + o * 128: k * 3 * C + o * 128 + 128],
                            xh[:, k * F:(k + 1) * F],
                            start=(k == 0), stop=(k == CH - 1),
                        )
                    oc = o % CH
                    if o < CH:
                        nc.vector.tensor_scalar(
                            qT[:, oc * F:(oc + 1) * F], ps[:, 0:F],
                            qkvb[:, o:o + 1], SCALE, op0=AL.add, op1=AL.mult)
                    else:
                        nc.vector.tensor_scalar(
                            kT[:, oc * F:(oc + 1) * F], ps[:, 0:F],
                            qkvb[:, o:o + 1], None, op0=AL.add)

                # v in token-partition layout, per image: 2 t-chunks
                vto = [[None, None], [None, None]]
                for b in range(2):
                    for tchunk in range(2):
                        tlen = mlens[tchunk]
                        toff = b * N + tchunk * 128
                        vt = tA.tile([128, C], f32r, tag=f"v{b}{tchunk}")
                        vto[b][tchunk] = vt
                        for half in range(2):
                            ps = psA()
                            for k in range(CH):
                                nc.tensor.matmul(
                                    ps[0:tlen, 0:384],
                                    xh[:, k * F + toff: k * F + toff + tlen],
                                    wq[:, k * 3 * C + 2 * C + half * 384:
                                       k * 3 * C + 2 * C + half * 384 + 384],
                                    start=(k == 0), stop=(k == CH - 1),
                                )
                            nc.vector.tensor_copy(
                                vt[0:tlen, half * 384:(half + 1) * 384],
                                ps[0:tlen, 0:384])

                # attention, per head; odd heads go via a staging tile +
                # SBUF->SBUF DMA (matmul outs must start at partition 0, and
                # DVE cannot shift partitions; DMA can).
                oT = tA.tile([128, CH * F], f32r, tag="oT")
                for h in range(HD):
                    hp, hh = h // 2, h % 2
                    qrow = hh * 64
                    qcol = hp * F
                    et = [tA.tile([128, F], f32r, tag=f"expT{i}", bufs=2,
                                  name=f"expT{i}")
                          for i in range(2)]
                    pev = psB() if l == SEL else None
                    for tchunk in range(2):
                        tlen = mlens[tchunk]
                        ps_b = [psA(), psA()]
                        for b in range(2):
                            nc.tensor.matmul(
                                ps_b[b][0:tlen, 0:F],
                                kT[qrow:qrow + 64,
                                   qcol + b * N + tchunk * 128:
                                   qcol + b * N + tchunk * 128 + tlen],
                                qT[qrow:qrow + 64, qcol:qcol + F],
                                start=True, stop=True,
                            )
                        if l == SEL:
                            rt = tA.tile([128, F], f32r, tag=f"reluT{tchunk}", bufs=1)
                            for b in range(2):
                                nc.vector.tensor_scalar(
                                    rt[0:tlen, b * N:(b + 1) * N],
                                    ps_b[b][0:tlen, b * N:(b + 1) * N],
                                    0.0, None, op0=AL.max)
                            nc.tensor.matmul(
                                pev[:, 0:F], ones[0:tlen, :], rt[0:tlen, 0:F],
                                start=(tchunk == 0), stop=(tchunk == 1),
                            )
                        for b in range(2):
                            nc.scalar.activation(
                                et[tchunk][0:tlen, b * N:(b + 1) * N],
                                ps_b[b][0:tlen, b * N:(b + 1) * N], AF.Exp)
                    if l == SEL:
                        ev1 = tA.tile([1, F], f32, tag="ev1")
                        nc.vector.tensor_scalar(
                            ev1[:], pev[0:1, 0:F], float(N), None, op0=AL.add)
                        nc.vector.reciprocal(ev1[:], ev1[:])
                        if h == 0:
                            nc.vector.tensor_copy(unc[p][:], ev1[:])
                        else:
                            nc.vector.tensor_tensor(
                                unc[p][:], ev1[:], unc[p][:], op=AL.add)
                    # softmax denominator (broadcast over 64 partitions)
                    prs = psC()
                    for tchunk in range(2):
                        tlen = mlens[tchunk]
                        nc.tensor.matmul(
                            prs[0:64, 0:F], ones[0:tlen, 0:64],
                            et[tchunk][0:tlen, 0:F],
                            start=(tchunk == 0), stop=(tchunk == 1),
                        )
                    rsb = tA.tile([64, F], f32, tag="rsb", bufs=2)
                    nc.vector.reciprocal(rsb[:], prs[0:64, 0:F])
                    # AV, one psum per image-version (other half discarded)
                    pav = [psB(), psB()]
                    for b in range(2):
                        for tchunk in range(2):
                            tlen = mlens[tchunk]
                            nc.tensor.matmul(
                                pav[b][0:64, 0:F],
                                vto[b][tchunk][0:tlen, h * 64:h * 64 + 64],
                                et[tchunk][0:tlen, 0:F],
                                start=(tchunk == 0), stop=(tchunk == 1),
                            )
                    # normalize; even heads land in rows 0:64 directly
                    if hh == 0:
                        for b in range(2):
                            cols = slice(hp * F + b * N, hp * F + (b + 1) * N)
                            pcols = slice(b * N, (b + 1) * N)
                            nc.vector.tensor_tensor(
                                oT[0:64, cols], pav[b][0:64, pcols],
                                rsb[0:64, pcols], op=AL.mult)
                    else:
                        stage = tA.tile([64, F], f32r, tag="avstage", bufs=2)
                        for b in range(2):
                            pcols = slice(b * N, (b + 1) * N)
                            nc.vector.tensor_tensor(
                                stage[0:64, pcols], pav[b][0:64, pcols],
                                rsb[0:64, pcols], op=AL.mult)
                        nc.sync.dma_start(
                            oT[64:128, hp * F:(hp + 1) * F], stage[0:64, 0:F])
                        # v-bias for the whole chunk, once per head pair
                        nc.vector.tensor_scalar(
                            oT[:, hp * F:(hp + 1) * F],
                            oT[:, hp * F:(hp + 1) * F].bitcast(f32),
                            qkvb[:, 12 + hp:13 + hp], None, op0=AL.add)

                # proj + residual
                for co in range(CH):
                    ps = psA()
                    for k in range(CH):
                        nc.tensor.matmul(
                            ps[:, 0:F],
                            wpj[:, k * C + co * 128: k * C + co * 128 + 128],
                            oT[:, k * F:(k + 1) * F],
                            start=(k == 0), stop=(k == CH - 1),
                        )
                    nc.vector.scalar_tensor_tensor(
                        xt[p][:, co * F:(co + 1) * F],
                        ps[:, 0:F], projb[:, co:co + 1],
                        xt[p][:, co * F:(co + 1) * F].bitcast(f32),
                        op0=AL.add, op1=AL.add)

        tap(1 + 2 * l, xt[0], F)

        # ---------------- pruning (after layer-SEL attention residual) --------
        if l == SEL:
            _prune(nc, tc, xt, unc, ident, ltt, iota, psB, psC, d)

        F = F0 if l < SEL else F1

        fc1b = load_bias(d["fc1b_d"], l, 24)
        fc2b = load_bias(d["fc2b_d"], l, 6)

        # ---------------- phase B: LN2 + MLP in 4 quarters ---------------------
        with tc.tile_pool(name="wB", bufs=1) as wB, tc.tile_pool(name="tB", bufs=1) as tB:
            xh2 = [layernorm(tB, xt[p], F, f"ln2_{p}") for p in range(PAIRS)]
            h1 = [tB.tile([128, CH * F], f32r, tag=f"h1_{p}", name=f"h1_{p}") for p in range(PAIRS)]
            for q in range(4):
                w1 = wB.tile([128, CH * C], f32r, tag="wfc1", bufs=1)
                nc.sync.dma_start(
                    w1[:].rearrange("p (k n) -> p k n", k=CH),
                    d["fc1w_d"][l][:, q * C:(q + 1) * C].rearrange("(k p) n -> p k n", p=128))
                w2 = wB.tile([128, CH * C], f32r, tag="wfc2", bufs=1)
                nc.sync.dma_start(
                    w2[:].rearrange("p (k n) -> p k n", k=CH),
                    d["fc2w_d"][l][q * C:(q + 1) * C, :].rearrange("(k p) n -> p k n", p=128))
                for p in range(PAIRS):
                    for co in range(CH):
                        ps = psA()
                        for k in range(CH):
                            nc.tensor.matmul(
                                ps[:, 0:F],
                                w1[:, k * C + co * 128: k * C + co * 128 + 128],
                                xh2[p][:, k * F:(k + 1) * F],
                                start=(k == 0), stop=(k == CH - 1),
                            )
                        nc.scalar.activation(
                            h1[p][:, co * F:(co + 1) * F], ps[:, 0:F],
                            AF.Gelu, bias=fc1b[:, q * CH + co:q * CH + co + 1])
                    for co in range(CH):
                        ps = psA()
                        for k in range(CH):
                            nc.tensor.matmul(
                                ps[:, 0:F],
                                w2[:, k * C + co * 128: k * C + co * 128 + 128],
                                h1[p][:, k * F:(k + 1) * F],
                                start=(k == 0), stop=(k == CH - 1),
                            )
                        if q == 0:
                            nc.vector.scalar_tensor_tensor(
                                xt[p][:, co * F:(co + 1) * F],
                                ps[:, 0:F], fc2b[:, co:co + 1],
                                xt[p][:, co * F:(co + 1) * F].bitcast(f32),
                                op0=AL.add, op1=AL.add)
                        else:
                            nc.vector.tensor_tensor(
                                xt[p][:, co * F:(co + 1) * F],
                                ps[:, 0:F],
                                xt[p][:, co * F:(co + 1) * F].bitcast(f32),
                                op=AL.add)
        tap(2 + 2 * l, xt[0], F)

    # ------------------------------------------------------------ head
    with tc.tile_pool(name="whead", bufs=1) as wh, tc.tile_pool(name="thead", bufs=1) as th:
        clsT = th.tile([128, CH * B_CORE], f32r, tag="clsT")
        for p in range(PAIRS):
            for b in range(2):
                for k in range(CH):
                    nc.vector.tensor_copy(
                        clsT[:, k * B_CORE + 2 * p + b: k * B_CORE + 2 * p + b + 1],
                        xt[p][:, k * F1 + b * N1: k * F1 + b * N1 + 1])
        xhc = layernorm(th, clsT, B_CORE, "lnf")
        hw = wh.tile([128, CH * NCLS], f32r, tag="hw")
        nc.sync.dma_start(hw[:].rearrange("p (k n) -> p k n", k=CH), d["headw_d"].rearrange("(k p) n -> p k n", p=128))
        hb = wh.tile([NCLS, 1], f32, tag="hb")
        nc.sync.dma_start(hb[:], d["headb_d"][:])
        ps = psC()
        for k in range(CH):
            nc.tensor.matmul(
                ps[0:NCLS, 0:B_CORE],
                hw[:, k * NCLS:(k + 1) * NCLS],
                xhc[:, k * B_CORE:(k + 1) * B_CORE],
                start=(k == 0), stop=(k == CH - 1),
            )
        lt = th.tile([NCLS, B_CORE], f32, tag="logits")
        nc.vector.tensor_scalar(lt[:], ps[0:NCLS, 0:B_CORE], hb[:, 0:1], None, op0=AL.add)
        nc.sync.dma_start(d["out_d"][:], lt[:])

    es.close()


def _prune(nc, tc, xt, unc, ident, ltt, iota, psB, psC, d):
    """Keep the K_KEEP lowest-uncertainty image tokens (drop the N_DROP
    highest), append mean of dropped; rewrite x in-place to [128, CH*F1]."""
    f32, f32r = dt.float32, dt.float32r
    jl = [128, 68]          # img-token chunk lengths (196 = 128 + 68)
    with tc.tile_pool(name="tprune", bufs=1) as tp:
        U = tp.tile([B_CORE, 196], f32, tag="U")
        for p in range(PAIRS):
            for b in range(2):
                # DVE writes must start at a 32-aligned partition; use DMA
                nc.sync.dma_start(
                    U[2 * p + b:2 * p + b + 1, :],
                    unc[p][:, b * N0 + 1:(b + 1) * N0])
        # drop mask: top-N_DROP largest per row (unc ~ 1, min_val 0 is safe;
        # mask threshold min(.,1) needs kept residuals >= 1?  values here are
        # sums of 12 reciprocals in (0,1): ~0.6..1.2 -- scale first to be safe.
        nc.vector.tensor_scalar(U[:], U[:], 100.0, None, op0=AL.mult)
        work = tp.tile([B_CORE, 196], f32, tag="work")
        mx = tp.tile([B_CORE, 8], f32, tag="mx")
        cur = U
        for k_on in range(0, N_DROP, 8):
            nfind = min(k_on + 8, N_DROP) - k_on
            nc.vector.max(out=mx[:], in_=cur[:])
            if nfind < 8:
                nc.vector.memset(mx[:, nfind:], 0.0)
            nc.vector.match_replace(out=work[:], in_to_replace=mx[:],
                                    in_values=cur[:], imm_value=0.0)
            cur = work
        nc.vector.tensor_sub(work[:], U[:], work[:])
        nc.vector.tensor_scalar_min(work[:], work[:], 1.0)   # drop mask {0,1}
        keep = tp.tile([B_CORE, 196], f32, tag="keep")
        nc.vector.tensor_scalar(keep[:], work[:], -1.0, 1.0, op0=AL.mult, op1=AL.add)
        if d.get("dbgp_d") is not None:
            nc.sync.dma_start(d["dbgp_d"][0][0:8, :], U[:])
            nc.sync.dma_start(d["dbgp_d"][1][0:8, :], keep[:])

        # keepT chunks via PE transpose
        keepT = [tp.tile([128, B_CORE], f32r, tag=f"keepT{i}", name=f"keepT{i}") for i in range(2)]
        for i in range(2):
            pt = psB()
            nc.tensor.transpose(pt[0:jl[i], 0:B_CORE],
                                keep[:, i * 128:i * 128 + jl[i]],
                                ident[0:B_CORE, 0:B_CORE])
            nc.vector.tensor_copy(keepT[i][0:jl[i], :], pt[0:jl[i], 0:B_CORE])
        # ranks = inclusive cumsum of keep via lower-triangular ones matmul
        prk = psC()
        for i in range(2):
            nc.tensor.matmul(
                prk[0:B_CORE, 0:196], keepT[i][0:jl[i], :],
                ltt[0:jl[i], i * 196:(i + 1) * 196],
                start=(i == 0), stop=(i == 1))
        ranks = tp.tile([B_CORE, 196], f32, tag="ranks")
        nc.vector.tensor_copy(ranks[:], prk[0:B_CORE, 0:196])
        if d.get("dbgp_d") is not None:
            nc.sync.dma_start(d["dbgp_d"][2][0:8, :], ranks[:])
        # target col t = keep*rank + (1-keep)*138 ; weight w = keep + (1-keep)/59
        tcol = tp.tile([B_CORE, 196], f32, tag="tcol")
        nc.vector.tensor_tensor(tcol[:], ranks[:], keep[:], op=AL.mult)
        nc.vector.scalar_tensor_tensor(tcol[:], keep[:], -float(N1 - 1), tcol[:],
                                       op0=AL.mult, op1=AL.add)
        nc.vector.tensor_scalar(tcol[:], tcol[:], float(N1 - 1), None, op0=AL.add)
        wcol = tp.tile([B_CORE, 196], f32, tag="wcol")
        nc.vector.tensor_scalar(wcol[:], keep[:], float((N_DROP - 1) / N_DROP),
                                1.0 / N_DROP, op0=AL.mult, op1=AL.add)
        tT = [tp.tile([128, B_CORE], f32, tag=f"tT{i}", name=f"tT{i}") for i in range(2)]
        wT = [tp.tile([128, B_CORE], f32, tag=f"wT{i}", name=f"wT{i}") for i in range(2)]
        for i in range(2):
            pt = psB()
            nc.tensor.transpose(pt[0:jl[i], 0:B_CORE],
                                tcol[:, i * 128:i * 128 + jl[i]],
                                ident[0:B_CORE, 0:B_CORE])
            nc.vector.tensor_copy(tT[i][0:jl[i], :], pt[0:jl[i], 0:B_CORE])
            pt2 = psB()
            nc.tensor.transpose(pt2[0:jl[i], 0:B_CORE],
                                wcol[:, i * 128:i * 128 + jl[i]],
                                ident[0:B_CORE, 0:B_CORE])
            nc.vector.tensor_copy(wT[i][0:jl[i], :], pt2[0:jl[i], 0:B_CORE])

        # per pair: transpose old x (img tokens only, cls-skipped so chunks
        # align with P), cls copies, then one-hot gather matmul, in place.
        for p in range(PAIRS):
            xa = xt[p]
            xtok = {}
            for b in range(2):
                for i in range(2):
                    tlen = jl[i]
                    xk = tp.tile([128, CH * 128], f32r, tag=f"xtok{b}{i}")
                    xtok[(b, i)] = xk
                    for k in range(CH):
                        pt = psB()
                        nc.tensor.transpose(
                            pt[0:tlen, 0:128],
                            xa[:, k * F0 + b * N0 + 1 + i * 128:
                               k * F0 + b * N0 + 1 + i * 128 + tlen].bitcast(f32),
                            ident[:])
                        nc.vector.tensor_copy(xk[0:tlen, k * 128:(k + 1) * 128],
                                              pt[0:tlen, 0:128])
            for b in range(2):
                for k in range(CH):
                    nc.vector.tensor_copy(
                        xa[:, k * F1 + b * N1: k * F1 + b * N1 + 1],
                        xa[:, k * F0 + b * N0: k * F0 + b * N0 + 1])
            for b in range(2):
                img = 2 * p + b
                P = [tp.tile([128, N1 - 1], f32r, tag=f"P{i}", name=f"P{i}") for i in range(2)]
                for i in range(2):
                    nc.vector.tensor_scalar(
                        P[i][0:jl[i], :], iota[0:jl[i], :],
                        tT[i][0:jl[i], img:img + 1], wT[i][0:jl[i], img:img + 1],
                        op0=AL.is_equal, op1=AL.mult)
                for k in range(CH):
                    pg = psC()
                    for i in range(2):
                        nc.tensor.matmul(
                            pg[0:128, 0:N1 - 1],
                            xtok[(b, i)][0:jl[i], k * 128:(k + 1) * 128],
                            P[i][0:jl[i], :],
                            start=(i == 0), stop=(i == 1))
                    nc.vector.tensor_copy(
                        xa[:, k * F1 + b * N1 + 1: k * F1 + b * N1 + N1],
                        pg[0:128, 0:N1 - 1])


# ------------------------------------------------------------------- host
def _host_pack(inputs):
    """Fold LN affines into weights, pre-transpose, pre-extract patches."""
    f = np.float32
    inp = {k: np.asarray(v, f) for k, v in inputs.items()}
    out = {}

    imgs = inp['inputs']
    B = imgs.shape[0]
    x = imgs.reshape(B, 3, 14, 16, 14, 16).transpose(0, 2, 4, 1, 3, 5).reshape(B, 196, 768)
    out['patchesT_full'] = np.ascontiguousarray(x.transpose(2, 0, 1).reshape(768, B * 196))

    posC = inp['pos_embed'][0].copy()
    posC[0] += inp['cls_token'][0, 0]
    posC[1:] += inp['patch_b'][None, :]
    out['posCT'] = np.ascontiguousarray(posC.T)

    out['patch_wT'] = np.ascontiguousarray(inp['patch_w'].reshape(C, -1).T)

    qkv_wT = np.empty((DEPTH, C, 3 * C), f)
    qkv_bL = np.empty((DEPTH, 128, 18), f)
    proj_wT = np.empty((DEPTH, C, C), f)
    proj_bL = np.empty((DEPTH, 128, 6), f)
    fc1_wT = np.empty((DEPTH, C, 4 * C), f)
    fc1_bL = np.empty((DEPTH, 128, 24), f)
    fc2_wT = np.empty((DEPTH, 4 * C, C), f)
    fc2_bL = np.empty((DEPTH, 128, 6), f)
    for l in range(DEPTH):
        w1 = inp['qkv_w'][l] * inp['ln1_g'][l][None, :]
        b1 = inp['qkv_b'][l] + inp['qkv_w'][l] @ inp['ln1_b'][l]
        qkv_wT[l] = w1.T
        qkv_bL[l] = b1.reshape(18, 128).T
        proj_wT[l] = inp['proj_w'][l].T
        proj_bL[l] = inp['proj_b'][l].reshape(6, 128).T
        wf1 = inp['fc1_w'][l] * inp['ln2_g'][l][None, :]
        bf1 = inp['fc1_b'][l] + inp['fc1_w'][l] @ inp['ln2_b'][l]
        fc1_wT[l] = wf1.T
        fc1_bL[l] = bf1.reshape(24, 128).T
        fc2_wT[l] = inp['fc2_w'][l].T
        fc2_bL[l] = inp['fc2_b'][l].reshape(6, 128).T
    out.update(qkv_wT=qkv_wT, qkv_bL=qkv_bL, proj_wT=proj_wT, proj_bL=proj_bL,
               fc1_wT=fc1_wT, fc1_bL=fc1_bL, fc2_wT=fc2_wT, fc2_bL=fc2_bL)

    hw = inp['head_w'] * inp['norm_g'][None, :]
    hb = inp['head_b'] + inp['head_w'] @ inp['norm_b']
    out['headT'] = np.ascontiguousarray(hw.T)
    out['head_bL'] = np.ascontiguousarray(hb.reshape(NCLS, 1))

    out['ident'] = np.eye(128, dtype=f)
    out['ones'] = np.ones((128, 128), f)
    out['invC'] = np.full((128, 128), 1.0 / C, f)
    out['iota'] = np.tile(np.arange(1, N1, dtype=f), (128, 1))
    out['LT'] = (np.arange(196)[:, None] <= np.arange(196)[None, :]).astype(f)
    return out


_BUILT = None


def kernel(**inputs):
    global _BUILT
    host = _host_pack(inputs)
    if _BUILT is None:
        nc = build_nc()
        split_excess_waits(nc)
        _BUILT = nc
    nc = _BUILT

    shared_keys = ['posCT', 'patch_wT', 'qkv_wT', 'qkv_bL', 'proj_wT', 'proj_bL',
                   'fc1_wT', 'fc1_bL', 'fc2_wT', 'fc2_bL', 'headT', 'head_bL',
                   'ident', 'ones', 'invC', 'iota', 'LT']
    in_maps = []
    for c in range(NCORES):
        m = {k: host[k] for k in shared_keys}
        m['patchesT'] = np.ascontiguousarray(
            host['patchesT_full'][:, c * B_CORE * 196:(c + 1) * B_CORE * 196])
        in_maps.append(m)

    trace = bool(os.environ.get("BASS_VIT_TRACE"))
    res = run_bass_kernel_spmd(nc, in_maps, core_ids=list(range(NCORES)), trace=trace)
    if trace:
        print(f"HW exec time: {res.exec_time_ns} ns (mean {res.mean_exec_time_ns})")
        kernel.last_exec_time_ns = res.exec_time_ns

    out = np.concatenate([res.results[c]["logitsT"].T for c in range(NCORES)],
                         axis=0).astype(np.float32)
    if os.environ.get("BASS_VIT_DEBUG_LAYER", ""):
        kernel.last_dbg = [res.results[c].get("dbg") for c in range(NCORES)]
        kernel.last_dbgp = [res.results[c].get("dbgp") for c in range(NCORES)]
    return out



# revision 27
# speedup vs baseline: 1.0784x; 1.0784x over previous
"""CertViT (ViT-Base + layer-3 token pruning) forward pass on 8 Trainium2 cores.

Data parallel: 8 images per core, processed as 4 image-pairs so matmul free
dims (394 / 278) stay >= 256 for full-rate fp32r. Activations live in
channel-partition layout x^T [768 -> 6x128 chunks, tokens]. LayerNorm affine
params are folded into the following matmul weights on the host; device LN is
pure standardization using ones-matmul partition broadcasts. Top-k pruning
uses max8/match_replace for the drop mask, a triangular-matmul cumsum for
ranks, and a one-hot permutation matmul for the gather.
"""

import os
import sys

import numpy as np

for _p in ('/opt/trn_rl_repo', '/root/.axon_site/_ro/trn_rl_repo'):
    if os.path.isdir(_p) and _p not in sys.path:
        sys.path.append(_p)

import concourse.bass as bass
import concourse.mybir as mybir
from concourse.tile import TileContext
from concourse.bass_utils import run_bass_kernel_spmd
from concourse.alu_op_type import AluOpType as AL

dt = mybir.dt
AF = mybir.ActivationFunctionType

# ---------------------------------------------------------------- config
NCORES = 8
B_CORE = 8            # images per core
PAIRS = B_CORE // 2
C = 768
CH = C // 128          # 6 channel chunks
HD = 12                # heads
D = 64                 # head dim
SCALE = D ** -0.5
DEPTH = 12
SEL = 3                # pruning layer
N0 = 197               # tokens before pruning
K_KEEP = 137           # int(197*0.7)
N_DROP = N0 - 1 - K_KEEP   # 59
N1 = K_KEEP + 2        # 139 tokens after pruning
F0 = 2 * N0            # pair free dim, layers 0..3
F1 = 2 * N1            # pair free dim, layers 4..11
EPS = 1e-6
NCLS = 100

# ------------------------------------------------------------- waitfix
# This walrus build accepts at most ONE sem wait per instruction; Tile can
# attach several. Move excess waits onto InstNoOp carriers inserted before.
_wf_counter = [0]


def _wf_carrier(engine, waits):
    _wf_counter[0] += 1
    d = mybir.InstNoOp(name=f"waitfix-{_wf_counter[0]}", ins=[], outs=[])
    d.engine = engine
    d.sync_info = mybir.SyncInfo(on_wait=list(waits), on_update=[])
    return d


def split_excess_waits(nc, max_waits=1):
    nfix = 0
    for f in nc.m.functions:
        for bb in f.blocks:
            insts = list(bb.instructions)
            out = []
            changed = False
            for inst in insts:
                si = inst.sync_info
                waits = list(si.on_wait) if si and si.on_wait else []
                if len(waits) > max_waits:
                    keep, rest = waits[:max_waits], waits[max_waits:]
                    while rest:
                        chunk, rest = rest[:max_waits], rest[max_waits:]
                        out.append(_wf_carrier(inst.engine, chunk))
                    si.on_wait = keep
                    changed = True
                    nfix += 1
                out.append(inst)
            if changed:
                bb.instructions = out
    return nfix


# ----------------------------------------------------------- device kernel
def build_nc():
    nc = bass.Bass()
    f32, f32r = dt.float32, dt.float32r

    d = {}
    d["patches_d"] = nc.declare_dram_parameter("patchesT", [C, B_CORE * 196], f32r, isOutput=False)
    d["posc_d"] = nc.declare_dram_parameter("posCT", [C, N0], f32, isOutput=False)
    d["pw_d"] = nc.declare_dram_parameter("patch_wT", [C, C], f32r, isOutput=False)
    d["qkvw_d"] = nc.declare_dram_parameter("qkv_wT", [DEPTH, C, 3 * C], f32r, isOutput=False)
    d["qkvb_d"] = nc.declare_dram_parameter("qkv_bL", [DEPTH, 128, 18], f32, isOutput=False)
    d["projw_d"] = nc.declare_dram_parameter("proj_wT", [DEPTH, C, C], f32r, isOutput=False)
    d["projb_d"] = nc.declare_dram_parameter("proj_bL", [DEPTH, 128, 6], f32, isOutput=False)
    d["fc1w_d"] = nc.declare_dram_parameter("fc1_wT", [DEPTH, C, 4 * C], f32r, isOutput=False)
    d["fc1b_d"] = nc.declare_dram_parameter("fc1_bL", [DEPTH, 128, 24], f32, isOutput=False)
    d["fc2w_d"] = nc.declare_dram_parameter("fc2_wT", [DEPTH, 4 * C, C], f32r, isOutput=False)
    d["fc2b_d"] = nc.declare_dram_parameter("fc2_bL", [DEPTH, 128, 6], f32, isOutput=False)
    d["headw_d"] = nc.declare_dram_parameter("headT", [C, NCLS], f32r, isOutput=False)
    d["headb_d"] = nc.declare_dram_parameter("head_bL", [NCLS, 1], f32, isOutput=False)
    d["ident_d"] = nc.declare_dram_parameter("ident", [128, 128], f32, isOutput=False)
    d["ones_d"] = nc.declare_dram_parameter("ones", [128, 128], f32r, isOutput=False)
    d["invc_d"] = nc.declare_dram_parameter("invC", [128, 128], f32r, isOutput=False)
    d["iota_d"] = nc.declare_dram_parameter("iota", [128, N1 - 1], f32, isOutput=False)
    d["lt_d"] = nc.declare_dram_parameter("LT", [196, 196], f32r, isOutput=False)
    d["out_d"] = nc.declare_dram_parameter("logitsT", [NCLS, B_CORE], f32, isOutput=True)

    d["dbg_layer"] = os.environ.get("BASS_VIT_DEBUG_LAYER", "")
    if d["dbg_layer"]:
        d["dbg_d"] = nc.declare_dram_parameter("dbg", [1 + 2 * DEPTH, 128, CH * F0], f32, isOutput=True)
        d["dbgp_d"] = nc.declare_dram_parameter("dbgp", [4, 8, 196], f32, isOutput=True)
    else:
        d["dbg_d"] = None
        d["dbgp_d"] = None

    with TileContext(nc) as tc:
        _build_body(nc, tc, d)
    return nc


def _build_body(nc, tc, d):
    f32, f32r = dt.float32, dt.float32r
    from contextlib import ExitStack
    es = ExitStack()

    cpool = es.enter_context(tc.tile_pool(name="consts", bufs=1))
    xpool = es.enter_context(tc.tile_pool(name="x", bufs=1))
    ppool = es.enter_context(tc.tile_pool(name="psum", bufs=1, space="PSUM"))
    prpool = es.enter_context(tc.tile_pool(name="prune", bufs=1))
    bpool = es.enter_context(tc.tile_pool(name="bias", bufs=2))

    # constants
    ident = cpool.tile([128, 128], f32, tag="ident")
    ones = cpool.tile([128, 128], f32r, tag="ones")
    invc = cpool.tile([128, 128], f32r, tag="invc")
    iota = cpool.tile([128, N1 - 1], f32, tag="iota")
    ltt = cpool.tile([128, 2 * 196], f32r, tag="ltt")
    posct = cpool.tile([128, CH * N0], f32, tag="posct")
    eps_t = cpool.tile([128, 1], f32, tag="eps_t")
    nc.vector.memset(eps_t[:], EPS)
    nc.sync.dma_start(ident[:], d["ident_d"][:])
    nc.sync.dma_start(ones[:], d["ones_d"][:])
    nc.sync.dma_start(invc[:], d["invc_d"][:])
    nc.sync.dma_start(iota[:], d["iota_d"][:])
    nc.sync.dma_start(ltt[:, 0:196], d["lt_d"][0:128, :])
    nc.sync.dma_start(ltt[0:68, 196:392], d["lt_d"][128:196, :])
    nc.sync.dma_start(posct[:].rearrange("p (k n) -> p k n", k=CH), d["posc_d"].rearrange("(k p) n -> p k n", p=128))

    # PSUM slots: tag 'a' x4 (main accumulations), 'b' x2, 'c' x2 -> 8 banks
    def psA():
        return ppool.tile([128, F0], f32, tag="a", bufs=4, name="psA")

    def psB():
        return ppool.tile([128, F0], f32, tag="b", bufs=2, name="psB")

    def psC():
        return ppool.tile([128, F0], f32, tag="c", bufs=2, name="psC")

    # persistent per-pair residual stream x^T, chunk-major [128, CH*F]
    xt = [xpool.tile([128, CH * F0], f32r, tag=f"x{p}", name=f"x{p}") for p in range(PAIRS)]
    # per-pair uncertainty rows (filled at layer SEL)
    unc = [prpool.tile([1, F0], f32, tag=f"unc{p}", name=f"unc{p}") for p in range(PAIRS)]

    # ------------------------------------------------------------ patch embed
    with tc.tile_pool(name="wpatch", bufs=1) as wp, tc.tile_pool(name="tpatch", bufs=2) as tp:
        pwt = wp.tile([128, CH * C], f32r, tag="pw")
        nc.sync.dma_start(pwt[:].rearrange("p (k n) -> p k n", k=CH), d["pw_d"].rearrange("(k p) n -> p k n", p=128))
        for p in range(PAIRS):
            prt = tp.tile([128, CH * 392], f32r, tag="patches")
            nc.sync.dma_start(
                prt[:].rearrange("p (k n) -> p k n", k=CH),
                d["patches_d"][:, p * 392:(p + 1) * 392].rearrange("(k p) n -> p k n", p=128),
            )
            for co in range(CH):
                ps = psA()
                for k in range(CH):
                    nc.tensor.matmul(
                        ps[:, 0:392],
                        pwt[:, k * C + co * 128: k * C + co * 128 + 128],
                        prt[:, k * 392:(k + 1) * 392],
                        start=(k == 0), stop=(k == CH - 1),
                    )
                for b in range(2):
                    nc.vector.tensor_tensor(
                        xt[p][:, co * F0 + b * N0 + 1: co * F0 + b * N0 + N0],
                        ps[:, b * 196:(b + 1) * 196],
                        posct[:, co * N0 + 1: co * N0 + N0],
                        op=AL.add,
                    )
                    nc.vector.tensor_copy(
                        xt[p][:, co * F0 + b * N0: co * F0 + b * N0 + 1],
                        posct[:, co * N0: co * N0 + 1],
                    )

    def tap(slot, xtile, F):
        if d["dbg_d"] is not None:
            nc.sync.dma_start(d["dbg_d"][slot][:, 0:CH * F], xtile[:, 0:CH * F].bitcast(f32))

    tap(0, xt[0], F0)

    # ------------------------------------------------------------ helpers
    def layernorm(pool, x, F, xh_tag, xh_bufs=1):
        """Standardize x (chunk-major [128, CH*F]) per token -> fp32r tile."""
        xh = pool.tile([128, CH * F], f32r, tag=xh_tag, bufs=xh_bufs, name=xh_tag)
        sq = pool.tile([128, CH * F], f32r, tag="ln_sq", bufs=1)
        for k in range(CH):
            nc.vector.tensor_tensor(
                sq[:, k * F:(k + 1) * F],
                x[:, k * F:(k + 1) * F].bitcast(f32),
                x[:, k * F:(k + 1) * F].bitcast(f32),
                op=AL.mult,
            )
        pm = psB()
        ps2 = psC()
        for k in range(CH):
            nc.tensor.matmul(pm[:, 0:F], invc[:], x[:, k * F:(k + 1) * F],
                             start=(k == 0), stop=(k == CH - 1))
        for k in range(CH):
            nc.tensor.matmul(ps2[:, 0:F], invc[:], sq[:, k * F:(k + 1) * F],
                             start=(k == 0), stop=(k == CH - 1))
        var = pool.tile([128, F], f32, tag="ln_var", bufs=1)
        rstd = pool.tile([128, F], f32, tag="ln_rstd", bufs=1)
        mean = pool.tile([128, F], f32, tag="ln_mean", bufs=1)
        nc.vector.tensor_copy(mean[:], pm[:, 0:F])
        nc.vector.tensor_tensor(var[:], mean[:], mean[:], op=AL.mult)
        nc.vector.tensor_tensor(var[:], ps2[:, 0:F], var[:], op=AL.subtract)
        nc.scalar.activation(rstd[:], var[:], AF.Ln, bias=eps_t[:, 0:1])
        nc.scalar.activation(rstd[:], rstd[:], AF.Exp, scale=-0.5)
        for k in range(CH):
            nc.vector.tensor_tensor(
                var[:], x[:, k * F:(k + 1) * F].bitcast(f32), mean[:], op=AL.subtract)
            nc.vector.tensor_tensor(
                xh[:, k * F:(k + 1) * F], var[:], rstd[:], op=AL.mult)
        return xh

    def load_bias(dram_t, l, cols):
        bt = bpool.tile([128, cols], f32, tag=dram_t.name)
        nc.sync.dma_start(bt[:], dram_t[l])
        return bt

    # ------------------------------------------------------------ layers
    for l in range(DEPTH):
        F = F0 if l <= SEL else F1
        N = N0 if l <= SEL else N1
        mlens = [128, N - 128]

        qkvb = load_bias(d["qkvb_d"], l, 18)
        projb = load_bias(d["projb_d"], l, 6)

        # ---------------- phase A: LN1 + QKV + attention + proj ----------------
        with tc.tile_pool(name="wA", bufs=1) as wA, tc.tile_pool(name="tA", bufs=1) as tA:
            wq = wA.tile([128, CH * 3 * C], f32r, tag="wqkv")
            nc.sync.dma_start(wq[:].rearrange("p (k n) -> p k n", k=CH), d["qkvw_d"][l].rearrange("(k p) n -> p k n", p=128))
            wpj = wA.tile([128, CH * C], f32r, tag="wproj")
            nc.sync.dma_start(wpj[:].rearrange("p (k n) -> p k n", k=CH), d["projw_d"][l].rearrange("(k p) n -> p k n", p=128))

            for p in range(PAIRS):
                xh = layernorm(tA, xt[p], F, "ln1")
                qT = tA.tile([128, CH * F], f32r, tag="qT")
                kT = tA.tile([128, CH * F], f32r, tag="kT")
                for o in range(12):
                    ps = psA()
                    for k in range(CH):
                        nc.tensor.matmul(
                            ps[:, 0:F],
                            wq[:, k * 3 * C + o * 128: k * 3 * C + o * 128 + 128],
                            xh[:, k * F:(k + 1) * F],
                            start=(k == 0), stop=(k == CH - 1),
                        )
                    oc = o % CH
                    if o < CH:
                        nc.vector.tensor_scalar(
                            qT[:, oc * F:(oc + 1) * F], ps[:, 0:F],
                            qkvb[:, o:o + 1], SCALE, op0=AL.add, op1=AL.mult)
                    else:
                        nc.vector.tensor_scalar(
                            kT[:, oc * F:(oc + 1) * F], ps[:, 0:F],
                            qkvb[:, o:o + 1], None, op0=AL.add)

                # v in token-partition layout, per image: 2 t-chunks
                vto = [[None, None], [None, None]]
                for b in range(2):
                    for tchunk in range(2):
                        tlen = mlens[tchunk]
                        toff = b * N + tchunk * 128
                        vt = tA.tile([128, C], f32r, tag=f"v{b}{tchunk}")
                        vto[b][tchunk] = vt
                        for half in range(2):
                            ps = psA()
                            for k in range(CH):
                                nc.tensor.matmul(
                                    ps[0:tlen, 0:384],
                                    xh[:, k * F + toff: k * F + toff + tlen],
                                    wq[:, k * 3 * C + 2 * C + half * 384:
                                       k * 3 * C + 2 * C + half * 384 + 384],
                                    start=(k == 0), stop=(k == CH - 1),
                                )
                            nc.vector.tensor_copy(
                                vt[0:tlen, half * 384:(half + 1) * 384],
                                ps[0:tlen, 0:384])

                # attention, per head; odd heads go via a staging tile +
                # SBUF->SBUF DMA (matmul outs must start at partition 0, and
                # DVE cannot shift partitions; DMA can).
                oT = tA.tile([128, CH * F], f32r, tag="oT")
                for h in range(HD):
                    hp, hh = h // 2, h % 2
                    qrow = hh * 64
                    qcol = hp * F
                    et = [tA.tile([128, F], f32r, tag=f"expT{i}", bufs=2,
                                  name=f"expT{i}")
                          for i in range(2)]
                    pev = psB() if l == SEL else None
                    for tchunk in range(2):
                        tlen = mlens[tchunk]
                        ps_b = [psA(), psA()]
                        for b in range(2):
                            nc.tensor.matmul(
                                ps_b[b][0:tlen, 0:F],
                                kT[qrow:qrow + 64,
                                   qcol + b * N + tchunk * 128:
                                   qcol + b * N + tchunk * 128 + tlen],
                                qT[qrow:qrow + 64, qcol:qcol + F],
                                start=True, stop=True,
                            )
                        if l == SEL:
                            rt = tA.tile([128, F], f32r, tag=f"reluT{tchunk}", bufs=1)
                            for b in range(2):
                                nc.vector.tensor_scalar(
                                    rt[0:tlen, b * N:(b + 1) * N],
                                    ps_b[b][0:tlen, b * N:(b + 1) * N],
                                    0.0, None, op0=AL.max)
                            nc.tensor.matmul(
                                pev[:, 0:F], ones[0:tlen, :], rt[0:tlen, 0:F],
                                start=(tchunk == 0), stop=(tchunk == 1),
                            )
                        for b in range(2):
                            nc.scalar.activation(
                                et[tchunk][0:tlen, b * N:(b + 1) * N],
                                ps_b[b][0:tlen, b * N:(b + 1) * N], AF.Exp)
                    if l == SEL:
                        ev1 = tA.tile([1, F], f32, tag="ev1")
                        nc.vector.tensor_scalar(
                            ev1[:], pev[0:1, 0:F], float(N), None, op0=AL.add)
                        nc.vector.reciprocal(ev1[:], ev1[:])
                        if h == 0:
                            nc.vector.tensor_copy(unc[p][:], ev1[:])
                        else:
                            nc.vector.tensor_tensor(
                                unc[p][:], ev1[:], unc[p][:], op=AL.add)
                    # softmax denominator (broadcast over 64 partitions)
                    prs = psC()
                    for tchunk in range(2):
                        tlen = mlens[tchunk]
                        nc.tensor.matmul(
                            prs[0:64, 0:F], ones[0:tlen, 0:64],
                            et[tchunk][0:tlen, 0:F],
                            start=(tchunk == 0), stop=(tchunk == 1),
                        )
                    rsb = tA.tile([64, F], f32, tag="rsb", bufs=2)
                    nc.scalar.activation(rsb[:], prs[0:64, 0:F], AF.Ln)
                    nc.scalar.activation(rsb[:], rsb[:], AF.Exp, scale=-1.0)
                    # AV, one psum per image-version (other half discarded)
                    pav = [psB(), psB()]
                    for b in range(2):
                        for tchunk in range(2):
                            tlen = mlens[tchunk]
                            nc.tensor.matmul(
                                pav[b][0:64, 0:F],
                                vto[b][tchunk][0:tlen, h * 64:h * 64 + 64],
                                et[tchunk][0:tlen, 0:F],
                                start=(tchunk == 0), stop=(tchunk == 1),
                            )
                    # normalize; even heads land in rows 0:64 directly
                    if hh == 0:
                        for b in range(2):
                            cols = slice(hp * F + b * N, hp * F + (b + 1) * N)
                            pcols = slice(b * N, (b + 1) * N)
                            nc.vector.tensor_tensor(
                                oT[0:64, cols], pav[b][0:64, pcols],
                                rsb[0:64, pcols], op=AL.mult)
                    else:
                        stage = tA.tile([64, F], f32r, tag="avstage", bufs=2)
                        for b in range(2):
                            pcols = slice(b * N, (b + 1) * N)
                            nc.vector.tensor_tensor(
                                stage[0:64, pcols], pav[b][0:64, pcols],
                                rsb[0:64, pcols], op=AL.mult)
                        nc.sync.dma_start(
                            oT[64:128, hp * F:(hp + 1) * F], stage[0:64, 0:F])
                        # v-bias for the whole chunk, once per head pair
                        nc.vector.tensor_scalar(
                            oT[:, hp * F:(hp + 1) * F],
                            oT[:, hp * F:(hp + 1) * F].bitcast(f32),
                            qkvb[:, 12 + hp:13 + hp], None, op0=AL.add)

                # proj + residual
                for co in range(CH):
                    ps = psA()
                    for k in range(CH):
                        nc.tensor.matmul(
                            ps[:, 0:F],
                            wpj[:, k * C + co * 128: k * C + co * 128 + 128],
                            oT[:, k * F:(k + 1) * F],
                            start=(k == 0), stop=(k == CH - 1),
                        )
                    nc.vector.scalar_tensor_tensor(
                        xt[p][:, co * F:(co + 1) * F],
                        ps[:, 0:F], projb[:, co:co + 1],
                        xt[p][:, co * F:(co + 1) * F].bitcast(f32),
                        op0=AL.add, op1=AL.add)

        tap(1 + 2 * l, xt[0], F)

        # ---------------- pruning (after layer-SEL attention residual) --------
        if l == SEL:
            _prune(nc, tc, xt, unc, ident, ltt, iota, psB, psC, d)

        F = F0 if l < SEL else F1

        fc1b = load_bias(d["fc1b_d"], l, 24)
        fc2b = load_bias(d["fc2b_d"], l, 6)

        # ---------------- phase B: LN2 + MLP in 4 quarters ---------------------
        with tc.tile_pool(name="wB", bufs=1) as wB, tc.tile_pool(name="tB", bufs=1) as tB:
            xh2 = [layernorm(tB, xt[p], F, f"ln2_{p}") for p in range(PAIRS)]
            h1 = [tB.tile([128, CH * F], f32r, tag=f"h1_{p}", name=f"h1_{p}") for p in range(PAIRS)]
            for q in range(4):
                w1 = wB.tile([128, CH * C], f32r, tag="wfc1", bufs=1)
                nc.sync.dma_start(
                    w1[:].rearrange("p (k n) -> p k n", k=CH),
                    d["fc1w_d"][l][:, q * C:(q + 1) * C].rearrange("(k p) n -> p k n", p=128))
                w2 = wB.tile([128, CH * C], f32r, tag="wfc2", bufs=1)
                nc.sync.dma_start(
                    w2[:].rearrange("p (k n) -> p k n", k=CH),
                    d["fc2w_d"][l][q * C:(q + 1) * C, :].rearrange("(k p) n -> p k n", p=128))
                for p in range(PAIRS):
                    for co in range(CH):
                        ps = psA()
                        for k in range(CH):
                            nc.tensor.matmul(
                                ps[:, 0:F],
                                w1[:, k * C + co * 128: k * C + co * 128 + 128],
                                xh2[p][:, k * F:(k + 1) * F],
                                start=(k == 0), stop=(k == CH - 1),
                            )
                        nc.scalar.activation(
                            h1[p][:, co * F:(co + 1) * F], ps[:, 0:F],
                            AF.Gelu, bias=fc1b[:, q * CH + co:q * CH + co + 1])
                    for co in range(CH):
                        ps = psA()
                        for k in range(CH):
                            nc.tensor.matmul(
                                ps[:, 0:F],
                                w2[:, k * C + co * 128: k * C + co * 128 + 128],
                                h1[p][:, k * F:(k + 1) * F],
                                start=(k == 0), stop=(k == CH - 1),
                            )
                        if q == 0:
                            nc.vector.scalar_tensor_tensor(
                                xt[p][:, co * F:(co + 1) * F],
                                ps[:, 0:F], fc2b[:, co:co + 1],
                                xt[p][:, co * F:(co + 1) * F].bitcast(f32),
                                op0=AL.add, op1=AL.add)
                        else:
                            nc.vector.tensor_tensor(
                                xt[p][:, co * F:(co + 1) * F],
                                ps[:, 0:F],
                                xt[p][:, co * F:(co + 1) * F].bitcast(f32),
                                op=AL.add)
        tap(2 + 2 * l, xt[0], F)

    # ------------------------------------------------------------ head
    with tc.tile_pool(name="whead", bufs=1) as wh, tc.tile_pool(name="thead", bufs=1) as th:
        clsT = th.tile([128, CH * B_CORE], f32r, tag="clsT")
        for p in range(PAIRS):
            for b in range(2):
                for k in range(CH):
                    nc.vector.tensor_copy(
                        clsT[:, k * B_CORE + 2 * p + b: k * B_CORE + 2 * p + b + 1],
                        xt[p][:, k * F1 + b * N1: k * F1 + b * N1 + 1])
        xhc = layernorm(th, clsT, B_CORE, "lnf")
        hw = wh.tile([128, CH * NCLS], f32r, tag="hw")
        nc.sync.dma_start(hw[:].rearrange("p (k n) -> p k n", k=CH), d["headw_d"].rearrange("(k p) n -> p k n", p=128))
        hb = wh.tile([NCLS, 1], f32, tag="hb")
        nc.sync.dma_start(hb[:], d["headb_d"][:])
        ps = psC()
        for k in range(CH):
            nc.tensor.matmul(
                ps[0:NCLS, 0:B_CORE],
                hw[:, k * NCLS:(k + 1) * NCLS],
                xhc[:, k * B_CORE:(k + 1) * B_CORE],
                start=(k == 0), stop=(k == CH - 1),
            )
        lt = th.tile([NCLS, B_CORE], f32, tag="logits")
        nc.vector.tensor_scalar(lt[:], ps[0:NCLS, 0:B_CORE], hb[:, 0:1], None, op0=AL.add)
        nc.sync.dma_start(d["out_d"][:], lt[:])

    es.close()


def _prune(nc, tc, xt, unc, ident, ltt, iota, psB, psC, d):
    """Keep the K_KEEP lowest-uncertainty image tokens (drop the N_DROP
    highest), append mean of dropped; rewrite x in-place to [128, CH*F1]."""
    f32, f32r = dt.float32, dt.float32r
    jl = [128, 68]          # img-token chunk lengths (196 = 128 + 68)
    with tc.tile_pool(name="tprune", bufs=1) as tp:
        U = tp.tile([B_CORE, 196], f32, tag="U")
        for p in range(PAIRS):
            for b in range(2):
                # DVE writes must start at a 32-aligned partition; use DMA
                nc.sync.dma_start(
                    U[2 * p + b:2 * p + b + 1, :],
                    unc[p][:, b * N0 + 1:(b + 1) * N0])
        # drop mask: top-N_DROP largest per row (unc ~ 1, min_val 0 is safe;
        # mask threshold min(.,1) needs kept residuals >= 1?  values here are
        # sums of 12 reciprocals in (0,1): ~0.6..1.2 -- scale first to be safe.
        nc.vector.tensor_scalar(U[:], U[:], 100.0, None, op0=AL.mult)
        work = tp.tile([B_CORE, 196], f32, tag="work")
        mx = tp.tile([B_CORE, 8], f32, tag="mx")
        cur = U
        for k_on in range(0, N_DROP, 8):
            nfind = min(k_on + 8, N_DROP) - k_on
            nc.vector.max(out=mx[:], in_=cur[:])
            if nfind < 8:
                nc.vector.memset(mx[:, nfind:], 0.0)
            nc.vector.match_replace(out=work[:], in_to_replace=mx[:],
                                    in_values=cur[:], imm_value=0.0)
            cur = work
        nc.vector.tensor_sub(work[:], U[:], work[:])
        nc.vector.tensor_scalar_min(work[:], work[:], 1.0)   # drop mask {0,1}
        keep = tp.tile([B_CORE, 196], f32, tag="keep")
        nc.vector.tensor_scalar(keep[:], work[:], -1.0, 1.0, op0=AL.mult, op1=AL.add)
        if d.get("dbgp_d") is not None:
            nc.sync.dma_start(d["dbgp_d"][0][0:8, :], U[:])
            nc.sync.dma_start(d["dbgp_d"][1][0:8, :], keep[:])

        # keepT chunks via PE transpose
        keepT = [tp.tile([128, B_CORE], f32r, tag=f"keepT{i}", name=f"keepT{i}") for i in range(2)]
        for i in range(2):
            pt = psB()
            nc.tensor.transpose(pt[0:jl[i], 0:B_CORE],
                                keep[:, i * 128:i * 128 + jl[i]],
                                ident[0:B_CORE, 0:B_CORE])
            nc.vector.tensor_copy(keepT[i][0:jl[i], :], pt[0:jl[i], 0:B_CORE])
        # ranks = inclusive cumsum of keep via lower-triangular ones matmul
        prk = psC()
        for i in range(2):
            nc.tensor.matmul(
                prk[0:B_CORE, 0:196], keepT[i][0:jl[i], :],
                ltt[0:jl[i], i * 196:(i + 1) * 196],
                start=(i == 0), stop=(i == 1))
        ranks = tp.tile([B_CORE, 196], f32, tag="ranks")
        nc.vector.tensor_copy(ranks[:], prk[0:B_CORE, 0:196])
        if d.get("dbgp_d") is not None:
            nc.sync.dma_start(d["dbgp_d"][2][0:8, :], ranks[:])
        # target col t = keep*rank + (1-keep)*138 ; weight w = keep + (1-keep)/59
        tcol = tp.tile([B_CORE, 196], f32, tag="tcol")
        nc.vector.tensor_tensor(tcol[:], ranks[:], keep[:], op=AL.mult)
        nc.vector.scalar_tensor_tensor(tcol[:], keep[:], -float(N1 - 1), tcol[:],
                                       op0=AL.mult, op1=AL.add)
        nc.vector.tensor_scalar(tcol[:], tcol[:], float(N1 - 1), None, op0=AL.add)
        wcol = tp.tile([B_CORE, 196], f32, tag="wcol")
        nc.vector.tensor_scalar(wcol[:], keep[:], float((N_DROP - 1) / N_DROP),
                                1.0 / N_DROP, op0=AL.mult, op1=AL.add)
        tT = [tp.tile([128, B_CORE], f32, tag=f"tT{i}", name=f"tT{i}") for i in range(2)]
        wT = [tp.tile([128, B_CORE], f32, tag=f"wT{i}", name=f"wT{i}") for i in range(2)]
        for i in range(2):
            pt = psB()
            nc.tensor.transpose(pt[0:jl[i], 0:B_CORE],
                                tcol[:, i * 128:i * 128 + jl[i]],
                                ident[0:B_CORE, 0:B_CORE])
            nc.vector.tensor_copy(tT[i][0:jl[i], :], pt[0:jl[i], 0:B_CORE])
            pt2 = psB()
            nc.tensor.transpose(pt2[0:jl[i], 0:B_CORE],
                                wcol[:, i * 128:i * 128 + jl[i]],
                                ident[0:B_CORE, 0:B_CORE])
            nc.vector.tensor_copy(wT[i][0:jl[i], :], pt2[0:jl[i], 0:B_CORE])

        # per pair: transpose old x (img tokens only, cls-skipped so chunks
        # align with P), cls copies, then one-hot gather matmul, in place.
        for p in range(PAIRS):
            xa = xt[p]
            xtok = {}
            for b in range(2):
                for i in range(2):
                    tlen = jl[i]
                    xk = tp.tile([128, CH * 128], f32r, tag=f"xtok{b}{i}")
                    xtok[(b, i)] = xk
                    for k in range(CH):
                        pt = psB()
                        nc.tensor.transpose(
                            pt[0:tlen, 0:128],
                            xa[:, k * F0 + b * N0 + 1 + i * 128:
                               k * F0 + b * N0 + 1 + i * 128 + tlen].bitcast(f32),
                            ident[:])
                        nc.vector.tensor_copy(xk[0:tlen, k * 128:(k + 1) * 128],
                                              pt[0:tlen, 0:128])
            for b in range(2):
                for k in range(CH):
                    nc.vector.tensor_copy(
                        xa[:, k * F1 + b * N1: k * F1 + b * N1 + 1],
                        xa[:, k * F0 + b * N0: k * F0 + b * N0 + 1])
            for b in range(2):
                img = 2 * p + b
                P = [tp.tile([128, N1 - 1], f32r, tag=f"P{i}", name=f"P{i}") for i in range(2)]
                for i in range(2):
                    nc.vector.tensor_scalar(
                        P[i][0:jl[i], :], iota[0:jl[i], :],
                        tT[i][0:jl[i], img:img + 1], wT[i][0:jl[i], img:img + 1],
                        op0=AL.is_equal, op1=AL.mult)
                for k in range(CH):
                    pg = psC()
                    for i in range(2):
                        nc.tensor.matmul(
                            pg[0:128, 0:N1 - 1],
                            xtok[(b, i)][0:jl[i], k * 128:(k + 1) * 128],
                            P[i][0:jl[i], :],
                            start=(i == 0), stop=(i == 1))
                    nc.vector.tensor_copy(
                        xa[:, k * F1 + b * N1 + 1: k * F1 + b * N1 + N1],
                        pg[0:128, 0:N1 - 1])


# ------------------------------------------------------------------- host
def _host_pack(inputs):
    """Fold LN affines into weights, pre-transpose, pre-extract patches."""
    f = np.float32
    inp = {k: np.asarray(v, f) for k, v in inputs.items()}
    out = {}

    imgs = inp['inputs']
    B = imgs.shape[0]
    x = imgs.reshape(B, 3, 14, 16, 14, 16).transpose(0, 2, 4, 1, 3, 5).reshape(B, 196, 768)
    out['patchesT_full'] = np.ascontiguousarray(x.transpose(2, 0, 1).reshape(768, B * 196))

    posC = inp['pos_embed'][0].copy()
    posC[0] += inp['cls_token'][0, 0]
    posC[1:] += inp['patch_b'][None, :]
    out['posCT'] = np.ascontiguousarray(posC.T)

    out['patch_wT'] = np.ascontiguousarray(inp['patch_w'].reshape(C, -1).T)

    qkv_wT = np.empty((DEPTH, C, 3 * C), f)
    qkv_bL = np.empty((DEPTH, 128, 18), f)
    proj_wT = np.empty((DEPTH, C, C), f)
    proj_bL = np.empty((DEPTH, 128, 6), f)
    fc1_wT = np.empty((DEPTH, C, 4 * C), f)
    fc1_bL = np.empty((DEPTH, 128, 24), f)
    fc2_wT = np.empty((DEPTH, 4 * C, C), f)
    fc2_bL = np.empty((DEPTH, 128, 6), f)
    for l in range(DEPTH):
        w1 = inp['qkv_w'][l] * inp['ln1_g'][l][None, :]
        b1 = inp['qkv_b'][l] + inp['qkv_w'][l] @ inp['ln1_b'][l]
        qkv_wT[l] = w1.T
        qkv_bL[l] = b1.reshape(18, 128).T
        proj_wT[l] = inp['proj_w'][l].T
        proj_bL[l] = inp['proj_b'][l].reshape(6, 128).T
        wf1 = inp['fc1_w'][l] * inp['ln2_g'][l][None, :]
        bf1 = inp['fc1_b'][l] + inp['fc1_w'][l] @ inp['ln2_b'][l]
        fc1_wT[l] = wf1.T
        fc1_bL[l] = bf1.reshape(24, 128).T
        fc2_wT[l] = inp['fc2_w'][l].T
        fc2_bL[l] = inp['fc2_b'][l].reshape(6, 128).T
    out.update(qkv_wT=qkv_wT, qkv_bL=qkv_bL, proj_wT=proj_wT, proj_bL=proj_bL,
               fc1_wT=fc1_wT, fc1_bL=fc1_bL, fc2_wT=fc2_wT, fc2_bL=fc2_bL)

    hw = inp['head_w'] * inp['norm_g'][None, :]
    hb = inp['head_b'] + inp['head_w'] @ inp['norm_b']
    out['headT'] = np.ascontiguousarray(hw.T)
    out['head_bL'] = np.ascontiguousarray(hb.reshape(NCLS, 1))

    out['ident'] = np.eye(128, dtype=f)
    out['ones'] = np.ones((128, 128), f)
    out['invC'] = np.full((128, 128), 1.0 / C, f)
    out['iota'] = np.tile(np.arange(1, N1, dtype=f), (128, 1))
    out['LT'] = (np.arange(196)[:, None] <= np.arange(196)[None, :]).astype(f)
    return out


_BUILT = None


def kernel(**inputs):
    global _BUILT
    host = _host_pack(inputs)
    if _BUILT is None:
        nc = build_nc()
        split_excess_waits(nc)
        _BUILT = nc
    nc = _BUILT

    shared_keys = ['posCT', 'patch_wT', 'qkv_wT', 'qkv_bL', 'proj_wT', 'proj_bL',
                   'fc1_wT', 'fc1_bL', 'fc2_wT', 'fc2_bL', 'headT', 'head_bL',
                   'ident', 'ones', 'invC', 'iota', 'LT']
    in_maps = []
    for c in range(NCORES):
        m = {k: host[k] for k in shared_keys}
        m['patchesT'] = np.ascontiguousarray(
            host['patchesT_full'][:, c * B_CORE * 196:(c + 1) * B_CORE * 196])
        in_maps.append(m)

    trace = bool(os.environ.get("BASS_VIT_TRACE"))
    res = run_bass_kernel_spmd(nc, in_maps, core_ids=list(range(NCORES)), trace=trace)
    if trace:
        print(f"HW exec time: {res.exec_time_ns} ns (mean {res.mean_exec_time_ns})")
        kernel.last_exec_time_ns = res.exec_time_ns

    out = np.concatenate([res.results[c]["logitsT"].T for c in range(NCORES)],
                         axis=0).astype(np.float32)
    if os.environ.get("BASS_VIT_DEBUG_LAYER", ""):
        kernel.last_dbg = [res.results[c].get("dbg") for c in range(NCORES)]
        kernel.last_dbgp = [res.results[c].get("dbgp") for c in range(NCORES)]
    return out

